# revision 4
# baseline (speedup 1.0000x reference)
"""GNN message-passing (mean aggregation + dual linear + relu + L2 norm)
on 8 Trainium2 NeuronCores.

v3: 2 gather blocks via signed-int16 window addressing (the gather ucode
sign-extends idx values and multiplies unsigned-stride-by-signed-idx, so a
window spans 65536 rows around a mid-table base).  Output tiles are
processed in block-0's vorder, so block-0 partial sums stay in SBUF (no
HBM roundtrip, no combine gather); only block-1 partials take the
HBM-partial + dma_gather path.  The CPU unpermutes rows at the end.

  - Nodes are globally sorted by in-degree and dealt round-robin to the 8
    cores, so every core runs the same compiled schedule (SPMD).
  - Device table [100003, 64]: zero row, h_neigh[0:65534], zero row,
    h_neigh[65534:100000], zero row.  Window A base = row 32768, window B
    base = row 98303; padding slots use a positive zero-row idx, and the
    final slot of every gather call is forced non-negative (the ucode
    strips trailing negative idxs).
  - Stage 1 (per block): nodes sorted by per-block edge count (within
    chunks) into 128-node vtiles; dma_gather fetches [128, K, 64] rows,
    DVE tensor_reduce sums the K slots.  Block-0 partials land in a
    per-chunk SBUF buffer; block-1 partials go to an HBM table.
  - Stage 2 (per chunk): one dma_gather pulls block-1 partials into
    block-0 vorder, DVE adds, then per tile: ACT 1/deg scale, PE
    transpose, W_neigh/W_self matmuls (PSUM-accumulated), ACT relu, PE
    transpose back, fused square+row-sum, sqrt, reciprocal, scale, DMA
    out (rows in block-0 vorder; CPU unpermutes).
  - CPU does only integer index prep (sorting, bucketing, permutations)
    plus input layout (transpose/permute of h_self, 1/deg) and the final
    row unpermute of the outputs.
"""
import numpy as np
import os as _os
from contextlib import ExitStack

N_NODES = 100000
N_EDGES = 1600000
D = 64
N_CORES = 8
NPC = 12544                 # nodes per core (98 tiles of 128)
NT = NPC // 128             # 98 output tiles per core
N_BLK = 2
BLK_SPLIT = 65534           # src < BLK_SPLIT -> block 0, else block 1
TBL_ROWS = N_NODES + 3
WIN_BASE = (32768, 98303)   # table row of idx==0 for each window
ZERO_IDX = (32767, 1699)    # positive in-window idx of a zero row
N_CHUNK = int(_os.environ.get("K_NCHUNK", "4"))
MAIN_CALL_IDX = int(_os.environ.get("K_CALL", "4096"))
SCRATCH = int(_os.environ.get("K_SCRATCH", "32768"))
GATH_BUFS = int(_os.environ.get("K_GBUFS", "6"))

_cache = {}


def _prep(h_neigh, h_self, src, dst, W_neigh, W_self):
    """CPU-side integer/index preprocessing. Returns (in_maps, schedule, meta)."""
    src = np.asarray(src, dtype=np.int64)
    dst = np.asarray(dst, dtype=np.int64)
    h_neigh = np.asarray(h_neigh, dtype=np.float32)
    h_self = np.asarray(h_self, dtype=np.float32)

    deg = np.bincount(dst, minlength=N_NODES)
    order = np.argsort(-deg, kind="stable")
    n_ext = NPC * N_CORES
    order_ext = np.concatenate(
        [order, np.full(n_ext - N_NODES, N_NODES, dtype=np.int64)])
    core_nodes = [order_ext[c::N_CORES] for c in range(N_CORES)]
    deg_ext = np.concatenate([deg, [0]])
    inv_deg = 1.0 / np.maximum(deg_ext, 1).astype(np.float32)

    e_order = np.argsort(dst, kind="stable")
    src_sorted = src[e_order]

    table = np.zeros((TBL_ROWS, D), dtype=np.float32)
    table[1:1 + BLK_SPLIT] = h_neigh[:BLK_SPLIT]
    table[BLK_SPLIT + 2:BLK_SPLIT + 2 + (N_NODES - BLK_SPLIT)] = h_neigh[BLK_SPLIT:]
    src_win_idx = np.where(
        np.arange(N_NODES) < BLK_SPLIT,
        np.arange(N_NODES) + 1 - WIN_BASE[0],
        np.arange(N_NODES) + 2 - WIN_BASE[1]).astype(np.int64)

    if N_CHUNK == 4:
        sizes = [27, 27, 27, 17]      # small last chunk shortens the tail
    else:
        base = NT // N_CHUNK
        rem = NT - base * N_CHUNK
        sizes = [base + (1 if i < rem else 0) for i in range(N_CHUNK)]
    chunks = []
    t = 0
    for w in sizes:
        chunks.append((t, t + w))
        t += w
    chunk_of_tile = np.zeros(NT, dtype=np.int64)
    for ch, (t0, t1) in enumerate(chunks):
        chunk_of_tile[t0:t1] = ch
    chunk_of_pos = np.repeat(chunk_of_tile, 128)
    chunk_base = np.array([t0 * 128 for (t0, t1) in chunks])

    per_core = []
    Ks = np.zeros((N_BLK, NT), dtype=np.int64)
    dstn = dst[e_order]
    rank_of_node = np.empty(N_NODES + 1, dtype=np.int64)
    rank_of_node[order_ext[:n_ext]] = np.arange(n_ext)
    e_rank = rank_of_node[dstn]
    e_core = e_rank % N_CORES
    e_pos = e_rank // N_CORES
    e_blk = (src_sorted >= BLK_SPLIT).astype(np.int64)
    e_widx = src_win_idx[src_sorted]

    for c in range(N_CORES):
        m = e_core == c
        cnts = np.zeros((N_BLK, NPC), dtype=np.int64)
        np.add.at(cnts, (e_blk[m], e_pos[m]), 1)
        pc = dict(nodes=core_nodes[c], cnts=cnts)
        # ascending widx per node so a node's last slot holds its max widx
        eo = np.lexsort((e_widx[m], e_pos[m], e_blk[m]))
        pc["edge_pos"] = e_pos[m][eo]
        pc["edge_blk"] = e_blk[m][eo]
        pc["edge_widx"] = e_widx[m][eo]
        maxw = np.full((N_BLK, NPC), -1, dtype=np.int64)
        np.maximum.at(maxw, (pc["edge_blk"], pc["edge_pos"]), pc["edge_widx"])
        pc["maxw"] = maxw
        vorders = []
        for b in range(N_BLK):
            vo = np.lexsort((-cnts[b], chunk_of_pos))
            vorders.append(vo)
            K = cnts[b][vo].reshape(NT, 128).max(axis=1)
            Ks[b] = np.maximum(Ks[b], K)
        pc["vorders"] = vorders
        per_core.append(pc)

    SK = Ks.sum(axis=1)
    col_base = np.zeros((N_BLK, NT + 1), dtype=np.int64)
    for b in range(N_BLK):
        col_base[b, 1:] = np.cumsum(Ks[b])
    calls = []                             # (ch, b, j0, j1, col0, ncols)
    for ch, (t0, t1) in enumerate(chunks):
        for b in range(N_BLK):
            j = t0
            while j < t1:
                j1 = j
                cols = 0
                while j1 < t1 and (cols == 0 or
                                   (cols + Ks[b][j1]) * 128 <= MAIN_CALL_IDX):
                    cols += Ks[b][j1]
                    j1 += 1
                calls.append((ch, b, j, j1, int(col_base[b, j]), int(cols)))
                j = j1
    sched = dict(Ks=Ks.tolist(), SK=SK.tolist(), calls=calls, chunks=chunks)

    # Fixup: the gather ucode strips trailing negative idxs from a call,
    # leaving those SBUF slots stale.  Swap a node whose final slot is
    # guaranteed non-negative into partition 127 of each call's last vtile.
    for c in range(N_CORES):
        pc = per_core[c]
        for (ch, b, j0, j1, col0, ncols) in calls:
            if ncols == 0:
                continue
            jlast = j1 - 1
            while Ks[b][jlast] == 0:
                jlast -= 1
            Kl = Ks[b][jlast]
            vo = pc["vorders"][b]
            seg = vo[jlast * 128:(jlast + 1) * 128]
            good = (pc["cnts"][b][seg] < Kl) | (pc["maxw"][b][seg] >= 0)
            if not good[127] and good.any():
                p = int(np.nonzero(good)[0][0])
                t = int(seg[127])
                seg[127] = seg[p]
                seg[p] = t

    in_maps = []
    Wn_T = np.ascontiguousarray(W_neigh.astype(np.float32).T)
    Ws_T = np.ascontiguousarray(W_self.astype(np.float32).T)
    wT = np.concatenate([Wn_T, Ws_T], axis=1)                   # [64, 128]
    h_self_ext = np.vstack([h_self, np.zeros((1, D), np.float32)])

    def wrap_cols(M):
        flat = M.T.reshape(-1)
        w = flat.reshape(-1, 16).T
        return np.tile(w, (8, 1)).copy()

    meta_nodes = []
    for c in range(N_CORES):
        pc = per_core[c]
        cnts = pc["cnts"]
        idx_main = []
        for b in range(N_BLK):
            vo = pc["vorders"][b]
            A = np.full((NT * 128, int(max(Ks[b].max(), 1))), ZERO_IDX[b],
                        dtype=np.int16)
            mb = pc["edge_blk"] == b
            epos = pc["edge_pos"][mb]
            ewidx = pc["edge_widx"][mb]
            vrank = np.empty(NPC, dtype=np.int64)
            vrank[vo] = np.arange(NPC)
            er = vrank[epos]
            so = np.argsort(er, kind="stable")
            er = er[so]
            ewidx_s = ewidx[so]
            slot = np.arange(er.size) - np.searchsorted(er, er)
            A[er, slot] = ewidx_s.astype(np.int16)
            SKb = int(SK[b])
            M = np.full((128, SKb), ZERO_IDX[b], dtype=np.int16)
            off = 0
            for j in range(NT):
                Kj = int(Ks[b][j])
                if Kj:
                    M[:, off:off + Kj] = A[j * 128:(j + 1) * 128, :Kj]
                off += Kj
            idx_main.append(wrap_cols(M))

        # combine idx for block 1: output slot r (= block-0 vorder slot r)
        # -> block-1 chunk-local vrank of that node
        vo0 = pc["vorders"][0]
        vo1 = pc["vorders"][1]
        vrank1 = np.empty(NPC, dtype=np.int64)
        vrank1[vo1] = np.arange(NPC)
        v = vrank1[vo0]                                 # [NPC] by output slot
        v_local = v - chunk_base[chunk_of_pos]          # chunk-local
        w = v_local.astype(np.int16).reshape(-1, 16).T
        idxc1 = np.tile(w, (8, 1)).copy()

        out_nodes = pc["nodes"][vo0]                    # output row r -> node
        meta_nodes.append(out_nodes)
        hsT = np.ascontiguousarray(h_self_ext[out_nodes].T)      # [64, NPC]
        ivd = inv_deg[out_nodes].reshape(NT, 128).T.copy()       # [128, NT]

        in_map = dict(tbl=table, hsT=hsT, ivd=ivd, wT=wT, idxc1=idxc1)
        for b in range(N_BLK):
            in_map[f"idxm{b}"] = idx_main[b]
        in_maps.append(in_map)

    meta = dict(out_nodes=meta_nodes)
    return in_maps, sched, meta


def _patch_queue_aware_sems():
    from concourse import tile_sem_assignment as tsa
    from concourse import mybir
    if getattr(tsa.TileClockTick, "_qaware_patched", False):
        return
    orig = tsa.TileClockTick._assign_tick

    def _assign_tick_qaware(self, inst):
        qn = getattr(inst, "queue_num", None)
        if qn is not None and getattr(inst, "engine", None) == mybir.EngineType.Pool:
            self.next_sw_dma_idx = int(qn) % self.swdge_sem_count
        return orig(self, inst)

    tsa.TileClockTick._assign_tick = _assign_tick_qaware
    tsa.TileClockTick._qaware_patched = True


def _build(sched):
    import concourse.bacc as bacc
    import concourse.tile as tile
    from concourse import mybir
    from concourse.masks import make_identity

    _patch_queue_aware_sems()

    F32 = mybir.dt.float32
    I16 = mybir.dt.int16
    AF = mybir.ActivationFunctionType
    Ks = np.array(sched["Ks"])
    SK = [int(x) for x in sched["SK"]]
    calls = sched["calls"]
    chunks = sched["chunks"]
    NTC = max(t1 - t0 for (t0, t1) in chunks)
    col_base = np.zeros((N_BLK, NT + 1), dtype=np.int64)
    for b in range(N_BLK):
        col_base[b, 1:] = np.cumsum(Ks[b])

    nc = bacc.Bacc("TRN2", target_bir_lowering=False, num_swdge_queues=4,
                   dynamic_dma_scratch_size=SCRATCH)
    tbl = nc.declare_dram_parameter("tbl", [TBL_ROWS, D], F32, isOutput=False)
    hsT = nc.declare_dram_parameter("hsT", [D, NPC], F32, isOutput=False)
    ivd = nc.declare_dram_parameter("ivd", [128, NT], F32, isOutput=False)
    wT = nc.declare_dram_parameter("wT", [D, 2 * D], F32, isOutput=False)
    idxm = [nc.declare_dram_parameter(f"idxm{b}", [128, SK[b] * 8], I16,
                                      isOutput=False) for b in range(N_BLK)]
    idxc1 = nc.declare_dram_parameter("idxc1", [128, NPC // 16], I16,
                                      isOutput=False)
    out = nc.declare_dram_parameter("out", [NPC, D], F32, isOutput=True)
    partial1 = [nc.dram_tensor(f"partial{ch}_1", [(t1 - t0) * 128, D], F32)
                for ch, (t0, t1) in enumerate(chunks)]

    with tile.TileContext(nc) as tc, ExitStack() as ctx:
        singles = ctx.enter_context(tc.tile_pool(name="singles", bufs=1))
        gp = ctx.enter_context(tc.tile_pool(name="gath", bufs=GATH_BUFS))
        p0 = ctx.enter_context(tc.tile_pool(name="part0", bufs=3))
        rp = ctx.enter_context(tc.tile_pool(name="red", bufs=6))
        cp = ctx.enter_context(tc.tile_pool(name="comb", bufs=2))
        wk = ctx.enter_context(tc.tile_pool(name="work", bufs=3))
        ps = ctx.enter_context(tc.tile_pool(name="psum", bufs=2, space="PSUM"))

        # idx loads: chunk 0's pieces first so stage 1 starts immediately
        idxm_sb = [singles.tile([128, SK[b] * 8], I16, name=f"idxm{b}_sb")
                   for b in range(N_BLK)]
        pieces = []
        for ch in range(len(chunks)):
            t0, t1 = chunks[ch]
            for b in range(N_BLK):
                c0 = int(col_base[b, t0]) * 8
                c1 = int(col_base[b, t1]) * 8
                if c1 > c0:
                    pieces.append((ch, b, c0, c1))

        def load_piece(ch):
            for (pch, b, c0, c1) in pieces:
                if pch == ch:
                    nc.sync.dma_start(out=idxm_sb[b][:, c0:c1],
                                      in_=idxm[b][:, c0:c1])

        load_piece(0)
        idxc1_sb = singles.tile([128, NPC // 16], I16)
        nc.sync.dma_start(out=idxc1_sb[:], in_=idxc1[:])
        hsT_sb = singles.tile([D, NPC], F32)
        nc.sync.dma_start(out=hsT_sb[:], in_=hsT[:])
        ivd_sb = singles.tile([128, NT], F32)
        nc.sync.dma_start(out=ivd_sb[:], in_=ivd[:])
        wT_sb = singles.tile([D, 2 * D], F32)
        nc.sync.dma_start(out=wT_sb[:], in_=wT[:])
        ident = singles.tile([128, 128], F32)
        make_identity(nc, ident[:])
        eps = singles.tile([128, 1], F32)
        nc.gpsimd.memset(eps[:], 1e-30)

        qn = [0]

        def next_q():
            q = qn[0] % 4
            qn[0] += 1
            return q

        pb0 = {}

        def stage1(ch):
            t0, t1 = chunks[ch]
            pb0[ch] = p0.tile([128, NTC, D], F32, tag="pb0", name="pb0")
            for (cch, b, j0, j1, col0, ncols) in calls:
                if cch != ch:
                    continue
                if ncols > 0:
                    g = gp.tile([128, ncols, D], F32, tag="g")
                    nc.gpsimd.dma_gather(
                        out_ap=g[:],
                        in_ap=tbl[WIN_BASE[b]:WIN_BASE[b] + 1, :],
                        idxs_ap=idxm_sb[b][:, col0 * 8:(col0 + ncols) * 8],
                        num_idxs=ncols * 128,
                        num_idxs_reg=ncols * 128,
                        elem_size=D,
                        single_packet=False,
                        queue_num=next_q(),
                    )
                off = 0
                for j in range(j0, j1):
                    Kj = int(Ks[b][j])
                    if b == 0:
                        red = pb0[ch][:, j - t0, :]
                    else:
                        red_t = rp.tile([128, D], F32, tag="red")
                        red = red_t[:]
                    if Kj == 0:
                        nc.vector.memset(red, 0.0)
                    elif Kj == 1:
                        nc.vector.tensor_copy(red, g[:, off, :])
                    else:
                        nc.vector.tensor_reduce(
                            out=red,
                            in_=g[:, off:off + Kj, :].rearrange("p k d -> p d k"),
                            axis=mybir.AxisListType.X,
                            op=mybir.AluOpType.add,
                        )
                    off += Kj
                    if b == 1:
                        nc.sync.dma_start(
                            out=partial1[ch][(j - t0) * 128:(j - t0 + 1) * 128, :],
                            in_=red)

        def stage2(ch):
            t0, t1 = chunks[ch]
            ntc = t1 - t0
            pb1 = cp.tile([128, ntc, D], F32, tag="pb1")
            nc.gpsimd.dma_gather(
                out_ap=pb1[:],
                in_ap=partial1[ch][:],
                idxs_ap=idxc1_sb[:, t0 * 8:t1 * 8],
                num_idxs=ntc * 128,
                num_idxs_reg=ntc * 128,
                elem_size=D,
                single_packet=False,
                queue_num=next_q(),
            )
            aggs = cp.tile([128, ntc, D], F32, tag="aggs")
            nc.vector.tensor_add(aggs[:], pb0[ch][:, 0:ntc, :], pb1[:])
            for ci in range(ntc):
                t = t0 + ci
                agg = wk.tile([128, D], F32, tag="agg")
                nc.scalar.mul(agg[:], aggs[:, ci, :], ivd_sb[:, t:t + 1])
                p_aT = ps.tile([D, 128], F32, tag="aT")
                nc.tensor.transpose(out=p_aT[:], in_=agg[:], identity=ident[:])
                aT = wk.tile([D, 128], F32, tag="aTs")
                nc.vector.tensor_copy(aT[:], p_aT[:])
                p_z = ps.tile([D, 128], F32, tag="z")
                nc.tensor.matmul(out=p_z[:], lhsT=wT_sb[:, 0:D], rhs=aT[:],
                                 start=True, stop=False)
                nc.tensor.matmul(out=p_z[:], lhsT=wT_sb[:, D:2 * D],
                                 rhs=hsT_sb[:, t * 128:(t + 1) * 128],
                                 start=False, stop=True)
                zT = wk.tile([D, 128], F32, tag="zT")
                nc.scalar.activation(zT[:], p_z[:], AF.Relu)
                p_zn = ps.tile([128, D], F32, tag="zn")
                nc.tensor.transpose(out=p_zn[:], in_=zT[:],
                                    identity=ident[0:D, 0:D])
                sq = wk.tile([128, D], F32, tag="sq")
                s = wk.tile([128, 1], F32, tag="s")
                nc.scalar.activation(sq[:], p_zn[:], AF.Square, accum_out=s[:])
                nrm = wk.tile([128, 1], F32, tag="nrm")
                nc.scalar.activation(nrm[:], s[:], AF.Sqrt, bias=eps[:])
                r = wk.tile([128, 1], F32, tag="r")
                nc.vector.reciprocal(r[:], nrm[:])
                o = wk.tile([128, D], F32, tag="o")
                nc.scalar.mul(o[:], p_zn[:], r[:])
                nc.sync.dma_start(out=out[t * 128:(t + 1) * 128, :], in_=o[:])

        with nc.named_scope("s1_0"):
            stage1(0)
        for ch in range(len(chunks)):
            if ch + 1 < len(chunks):
                load_piece(ch + 1)
                with nc.named_scope(f"s1_{ch + 1}"):
                    stage1(ch + 1)
            with nc.named_scope(f"s2_{ch}"):
                stage2(ch)

    nc.compile()
    return nc


def kernel(h_neigh, h_self, src, dst, W_neigh, W_self):
    from concourse.bass_utils import run_bass_kernel_spmd

    in_maps, sched, meta = _prep(h_neigh, h_self, src, dst, W_neigh, W_self)
    key = str(sched["Ks"])
    if key not in _cache:
        _cache[key] = _build(sched)
    nc = _cache[key]

    import os
    trace = bool(int(os.environ.get("KERNEL_TRACE", "0")))
    res = run_bass_kernel_spmd(nc, in_maps, core_ids=list(range(N_CORES)),
                               trace=trace)
    kernel.last_exec_time_ns = res.exec_time_ns
    kernel.last_result = res

    out = np.zeros((N_NODES, D), dtype=np.float32)
    for c in range(N_CORES):
        nodes = meta["out_nodes"][c]
        dev = res.results[c]["out"]                   # [NPC, 64]
        valid = nodes < N_NODES
        out[nodes[valid]] = dev[valid]
    return out


def last_exec_time_ns():
    return getattr(kernel, "last_exec_time_ns", None)


kernel.last_result = None
